# revision 37
# baseline (speedup 1.0000x reference)
"""Attention-decoder (B=128, T=256, F=512, O=512, MID=1000, 32 steps) on 8 trn2 cores.

Strategy: data-parallel over batch (16 per core). pre = a @ W1a.T + b1 is
computed once and kept in SBUF as fp16 [MID_p, (b,t)]. Each step:
  u = W1s @ s.T (PE, psum [128m, 16b] x8)          -> tiny
  hpre = pre + u (DVE tensor_scalar, per (mc,b))    -> fp16 4x mode
  h = tanh(hpre) (Act, one [128, 4096] inst per mc) -> act bottleneck
  logitsT[t,b] (PE: stationary=h slices, moving=W2 col, 1-col matmuls)
  softmax over t=partitions: relu+b2 (DVE), exp (Act, f32, no max-sub),
  ssum via ones-stationary matmul, recip (DVE), recip broadcast (PE)
  ctxT[f,b] (PE: stationary=aN tiles, moving=e col), normalize+fp16 (DVE)
  gates gT[o,b] (PE: stationary=WgT, moving=sT/ctxT chunks, bias rank-1)
  LSTM pointwise entirely in transposed [o-part, (oc,b)] layout; s stays
  transposed so the next step needs no transposes at all.
All matmuls keep the big operand stationary (free) and outputs skinny.
"""
import sys
import numpy as np

sys.path.insert(0, "/opt/trn_rl_repo")

DEBUG = False
B, T, F, O, MID = 128, 256, 512, 512, 1000
MIDP = 1024  # padded
NCORES = 8
BC = B // NCORES  # 16 batch per core
BT = BC * T       # 4096


def _build(wo: int):
    import concourse.bass as bass
    import concourse.bacc as bacc
    import concourse.mybir as mybir
    from concourse.tile import TileContext

    f16 = mybir.dt.float16
    f32 = mybir.dt.float32
    AF = mybir.ActivationFunctionType
    OP = mybir.AluOpType

    nc = bacc.Bacc()
    aT_d = nc.dram_tensor("aT", [F, BT], f16, kind="ExternalInput")
    aN_d = nc.dram_tensor("aN", [BT, F], f16, kind="ExternalInput")
    W1aT_d = nc.dram_tensor("W1aT", [F, MIDP], f16, kind="ExternalInput")
    W1sT_d = nc.dram_tensor("W1sT", [O, MIDP], f16, kind="ExternalInput")
    W2c_d = nc.dram_tensor("W2c", [128, 8], f16, kind="ExternalInput")
    b1T_d = nc.dram_tensor("b1T", [128, 8], f32, kind="ExternalInput")
    b2bc_d = nc.dram_tensor("b2bc", [128, 1], f32, kind="ExternalInput")
    WgT_d = nc.dram_tensor("WgT", [O + F, 4 * O], f16, kind="ExternalInput")
    bgT_d = nc.dram_tensor("bgT128", [128, 128], f32, kind="ExternalInput")
    sPT_d = nc.dram_tensor("sPT64", [2, 128, 32], f16, kind="ExternalInput")
    out_d = nc.dram_tensor("out", [wo, 2, 128, 32], f32, kind="ExternalOutput")

    with TileContext(nc) as tc:
        with (
            tc.tile_pool(name="const", bufs=1) as cp,
            tc.tile_pool(name="hpre", bufs=2) as hp,
            tc.tile_pool(name="hh", bufs=2) as hhp,
            tc.tile_pool(name="astream", bufs=2) as app,
            tc.tile_pool(name="strans", bufs=2) as stp,
            tc.tile_pool(name="work", bufs=2) as wp,
            tc.tile_pool(name="small", bufs=2) as sp,
            tc.tile_pool(name="ppre", bufs=2, space="PSUM") as ppre,
            tc.tile_pool(name="pw", bufs=1, space="PSUM") as pwp,
            tc.tile_pool(name="pa", bufs=1, space="PSUM") as pap,
            tc.tile_pool(name="pg", bufs=1, space="PSUM") as pgp,
        ):
            dma = nc.sync.dma_start

            # ---- constant loads ----
            aN_sb = {}
            for b in range(BC):
                for tcn in range(2):
                    t_ = cp.tile([128, F], f16, tag=f"aN{b}_{tcn}", name=f"aN{b}_{tcn}")
                    dma(t_[:], aN_d[b * T + tcn * 128: b * T + (tcn + 1) * 128, :])
                    aN_sb[(b, tcn)] = t_
            W1aT_sb, W1sT_sb, WgT_sb = [], [], []
            for kc in range(4):
                t_ = cp.tile([128, MIDP], f16, tag=f"w1a{kc}", name=f"w1a{kc}")
                dma(t_[:], W1aT_d[kc * 128:(kc + 1) * 128, :])
                W1aT_sb.append(t_)
            for kc in range(4):
                t_ = cp.tile([128, MIDP], f16, tag=f"w1s{kc}", name=f"w1s{kc}")
                dma(t_[:], W1sT_d[kc * 128:(kc + 1) * 128, :])
                W1sT_sb.append(t_)
            for kc in range(8):
                t_ = cp.tile([128, 4 * O], f16, tag=f"wg{kc}", name=f"wg{kc}")
                dma(t_[:], WgT_d[kc * 128:(kc + 1) * 128, :])
                WgT_sb.append(t_)
            W2_sb = cp.tile([128, 8], f16, tag="w2", name="w2")
            dma(W2_sb[:], W2c_d[:])
            b1T_sb = cp.tile([128, 8], f32, tag="b1t", name="b1t")
            dma(b1T_sb[:], b1T_d[:])
            b2bc_sb = cp.tile([128, 1], f32, tag="b2bc", name="b2bc")
            dma(b2bc_sb[:], b2bc_d[:])
            bgT_sb = cp.tile([128, 128], f32, tag="bgT", name="bgT")
            dma(bgT_sb[:], bgT_d[:])
            onesf = cp.tile([128, 128], f32, tag="onesf", name="onesf")
            nc.vector.memset(onesf[:], 1.0)

            # ---- precompute pre = (a @ W1a.T).T + b1 : [MID_p, (b,t)] fp16 ----
            pre_sb = []
            for mc in range(8):
                pre_sb.append(cp.tile([128, BT], f16, tag=f"pre{mc}", name=f"pre{mc}"))
            for ns in range(8):
                a_sl = []
                for kc in range(4):
                    t_ = app.tile([128, 512], f16, tag=f"astr{kc}", name=f"astr{kc}")
                    dma(t_[:], aT_d[kc * 128:(kc + 1) * 128, ns * 512:(ns + 1) * 512])
                    a_sl.append(t_)
                for mc in range(8):
                    ps = ppre.tile([128, 512], f32, tag="ppre", name="ppre")
                    for kc in range(4):
                        nc.tensor.matmul(
                            ps[:],
                            W1aT_sb[kc][:, mc * 128:(mc + 1) * 128],
                            a_sl[kc][:],
                            start=(kc == 0), stop=(kc == 3),
                        )
                    dst = pre_sb[mc][:, ns * 512:(ns + 1) * 512]
                    if mc % 2 == 0:
                        nc.scalar.activation(dst, ps[:], AF.Identity,
                                             bias=b1T_sb[:, mc:mc + 1], scale=1.0)
                    else:
                        nc.vector.tensor_scalar(
                            out=dst, in0=ps[:], scalar1=b1T_sb[:, mc:mc + 1],
                            scalar2=None, op0=OP.add)

            # ---- decode steps: two batch groups of 8, software-pipelined
            # so each group's serial tail hides under the other group's
            # tanh phase. All per-step psum lives in merged per-group banks:
            #   work_g [128,200] f32: u cols 0:64 (mc*8+bl), logit partials
            #     cols 64:192 (64+mc*16+tc*8+bl), softmax sum row cols 192:200
            #   att_g [128,48] f32: recip-bcast cols 0:16, ctx cols 16:48
            #   gate_g [128,256] f32: s-half cols 0:128, ctx-half 128:256
            BL = 8  # batches per group

            state = {}
            for g in (0, 1):
                sTg = stp.tile([128, 32], f16, tag=f"sT{g}", name=f"sT{g}_0")
                dma(sTg[:], sPT_d[g])
                cTg = wp.tile([128, 32], f32, tag=f"cT{g}", name=f"cT{g}_0")
                nc.vector.memset(cTg[:], 0.0)
                state[g] = {"sT": sTg, "cT": cTg}

            def head(g):
                st = state[g]
                w = pwp.tile([128, 200], f32, tag=f"w{g}", name=f"w{g}")
                for mc in range(8):
                    for kc in range(4):
                        nc.tensor.matmul(
                            w[:, mc * BL:(mc + 1) * BL],
                            W1sT_sb[kc][:, mc * 128:(mc + 1) * 128],
                            st["sT"][:, kc * BL:(kc + 1) * BL],
                            start=(kc == 0), stop=(kc == 3),
                        )
                usb = sp.tile([128, 64], f32, tag=f"usb{g}", name=f"usb{g}")
                nc.vector.tensor_copy(usb[:], w[:, 0:64])
                gt = pgp.tile([128, 256], f32, tag=f"g{g}", name=f"g{g}")
                for g4 in range(4):
                    for oc in range(4):
                        col = g4 * 32 + oc * BL
                        for kc in range(4):
                            nc.tensor.matmul(
                                gt[:, col:col + BL],
                                WgT_sb[kc][:, g4 * O + oc * 128: g4 * O + (oc + 1) * 128],
                                st["sT"][:, kc * BL:(kc + 1) * BL],
                                start=(kc == 0), stop=(kc == 3),
                            )
                gS = sp.tile([128, 128], f32, tag=f"gS{g}", name=f"gS{g}")
                nc.vector.tensor_tensor(out=gS[:], in0=gt[:, 0:128],
                                        in1=bgT_sb[:], op=OP.add)
                st.update(w=w, usb=usb, gate=gt, gS=gS)

            def chunk(g, mc):
                st = state[g]
                b0 = g * BL
                w = st["w"]
                hpre = hp.tile([128, BL * T], f16, tag=f"hpre{g}", name=f"hpre{g}")
                for bl in range(BL):
                    src = pre_sb[mc][:, (b0 + bl) * T:(b0 + bl + 1) * T]
                    dst = hpre[:, bl * T:(bl + 1) * T]
                    if bl < 6:
                        nc.vector.tensor_scalar(
                            out=dst, in0=src,
                            scalar1=w[:, mc * BL + bl: mc * BL + bl + 1],
                            scalar2=None, op0=OP.add)
                    else:
                        nc.gpsimd.tensor_scalar(
                            out=dst, in0=src,
                            scalar1=st["usb"][:, mc * BL + bl: mc * BL + bl + 1],
                            scalar2=None, op0=OP.add)
                h = hhp.tile([128, BL * T], f16, tag=f"h{g}", name=f"h{g}")
                nc.scalar.activation(h[:], hpre[:], AF.Tanh)
                for bl in range(BL):
                    for tcn in range(2):
                        col = 64 + mc * 16 + tcn * BL + bl
                        nc.tensor.matmul(
                            w[:, col:col + 1],
                            h[:, bl * T + tcn * 128: bl * T + (tcn + 1) * 128],
                            W2_sb[:, mc:mc + 1],
                            start=True, stop=True,
                        )

            def tailA(g):
                st = state[g]
                w = st["w"]
                lcp = sp.tile([128, 128], f32, tag=f"lcp{g}", name=f"lcp{g}")
                nc.vector.tensor_copy(lcp[:], w[:, 64:192])
                ra = sp.tile([128, 64], f32, tag=f"ra{g}", name=f"ra{g}")
                nc.vector.tensor_tensor(out=ra[:], in0=lcp[:, 0:64],
                                        in1=lcp[:, 64:128], op=OP.add)
                rb = sp.tile([128, 32], f32, tag=f"rb{g}", name=f"rb{g}")
                nc.vector.tensor_tensor(out=rb[:], in0=ra[:, 0:32],
                                        in1=ra[:, 32:64], op=OP.add)
                rc = sp.tile([128, 16], f32, tag=f"rc{g}", name=f"rc{g}")
                nc.vector.tensor_tensor(out=rc[:], in0=rb[:, 0:16],
                                        in1=rb[:, 16:32], op=OP.add)
                lg = sp.tile([128, 16], f32, tag=f"lg{g}", name=f"lg{g}")
                nc.vector.tensor_scalar(
                    out=lg[:], in0=rc[:], scalar1=b2bc_sb[:, 0:1],
                    scalar2=0.0, op0=OP.add, op1=OP.max,
                )
                e = sp.tile([128, 16], f32, tag=f"e{g}", name=f"e{g}")
                nc.scalar.activation(e[:], lg[:], AF.Exp)
                srow = w[0:1, 192:200]
                for tcn in range(2):
                    nc.tensor.matmul(
                        srow, onesf[:, 0:1], e[:, tcn * BL:(tcn + 1) * BL],
                        start=(tcn == 0), stop=(tcn == 1),
                    )
                rrow = sp.tile([1, 8], f32, tag=f"rr{g}", name=f"rr{g}")
                nc.vector.reciprocal(rrow[:], srow)
                att = pap.tile([128, 48], f32, tag=f"a{g}", name=f"a{g}")
                for tcn in range(2):
                    nc.tensor.matmul(att[:, tcn * BL:(tcn + 1) * BL],
                                     onesf[0:1, :], rrow[:],
                                     start=True, stop=True)
                alphT = sp.tile([128, 16], f16, tag=f"alphT{g}", name=f"alphT{g}")
                nc.vector.tensor_tensor(out=alphT[:], in0=e[:], in1=att[:, 0:16],
                                        op=OP.mult)
                st.update(att=att, alphT=alphT)

            def tailB(g):
                st = state[g]
                b0 = g * BL
                att, alphT, gt = st["att"], st["alphT"], st["gate"]
                for bl in range(BL):
                    for fc in range(4):
                        for tcn in range(2):
                            nc.tensor.matmul(
                                att[:, 16 + fc * BL + bl: 16 + fc * BL + bl + 1],
                                aN_sb[(b0 + bl, tcn)][:, fc * 128:(fc + 1) * 128],
                                alphT[:, tcn * BL + bl: tcn * BL + bl + 1],
                                start=(tcn == 0), stop=(tcn == 1),
                            )
                ctxT = stp.tile([128, 32], f16, tag=f"ctxT{g}", name=f"ctxT{g}")
                nc.vector.tensor_copy(ctxT[:], att[:, 16:48])
                for g4 in range(4):
                    for oc in range(4):
                        col = 128 + g4 * 32 + oc * BL
                        for kc in range(4, 8):
                            nc.tensor.matmul(
                                gt[:, col:col + BL],
                                WgT_sb[kc][:, g4 * O + oc * 128: g4 * O + (oc + 1) * 128],
                                ctxT[:, (kc - 4) * BL:(kc - 3) * BL],
                                start=(kc == 4), stop=(kc == 7),
                            )
                gsum = sp.tile([128, 128], f32, tag=f"gsum{g}", name=f"gsum{g}")
                nc.vector.tensor_tensor(out=gsum[:], in0=gt[:, 128:256],
                                        in1=st["gS"][:], op=OP.add)
                cand = sp.tile([128, 32], f32, tag=f"cand{g}", name=f"cand{g}")
                nc.scalar.activation(cand[:], gsum[:, 0:32], AF.Tanh)
                tT = sp.tile([128, 96], f32, tag=f"tT{g}", name=f"tT{g}")
                nc.scalar.activation(tT[:], gsum[:, 32:128], AF.Tanh, scale=0.5)
                st.update(cand=cand, tT=tT)

            def tailC(g, t):
                st = state[g]
                sig = sp.tile([128, 96], f32, tag=f"sig{g}", name=f"sig{g}")
                nc.vector.tensor_scalar(out=sig[:], in0=st["tT"][:], scalar1=0.5,
                                        scalar2=0.5, op0=OP.mult, op1=OP.add)
                t1 = sp.tile([128, 32], f32, tag=f"t1{g}", name=f"t1{g}")
                nc.vector.tensor_tensor(out=t1[:], in0=sig[:, 0:32],
                                        in1=st["cand"][:], op=OP.mult)
                t2 = sp.tile([128, 32], f32, tag=f"t2{g}", name=f"t2{g}")
                nc.gpsimd.tensor_tensor(out=t2[:], in0=sig[:, 32:64],
                                        in1=st["cT"][:], op=OP.mult)
                cT_new = wp.tile([128, 32], f32, tag=f"cT{g}", name=f"cT{g}")
                nc.vector.tensor_tensor(out=cT_new[:], in0=t1[:], in1=t2[:],
                                        op=OP.add)
                tch = sp.tile([128, 32], f32, tag=f"tch{g}", name=f"tch{g}")
                nc.scalar.activation(tch[:], cT_new[:], AF.Tanh)
                sT_new = stp.tile([128, 32], f16, tag=f"sT{g}", name=f"sT{g}")
                nc.vector.tensor_tensor(out=sT_new[:], in0=sig[:, 64:96],
                                        in1=tch[:], op=OP.mult)
                sOut = wp.tile([128, 32], f32, tag=f"sOut{g}", name=f"sOut{g}")
                nc.gpsimd.tensor_tensor(out=sOut[:], in0=sig[:, 64:96],
                                        in1=tch[:], op=OP.mult)
                dma(out_d[t, g, :, :], sOut[:])
                st["cT"] = cT_new
                st["sT"] = sT_new

            pending = None
            for t in range(wo):
                for g in (0, 1):
                    head(g)
                    chunk(g, 0)
                    chunk(g, 1)
                    if pending is not None:
                        tailA(pending[1])
                    chunk(g, 2)
                    chunk(g, 3)
                    if pending is not None:
                        tailB(pending[1])
                    chunk(g, 4)
                    chunk(g, 5)
                    if pending is not None:
                        tailC(pending[1], pending[0])
                    chunk(g, 6)
                    chunk(g, 7)
                    pending = (t, g)
            tailA(pending[1])
            tailB(pending[1])
            tailC(pending[1], pending[0])
    nc.compile()
    return nc


def _make_runner(nc):
    """Build the sharded jit callable ONCE per module (run_bass_via_pjrt
    rebuilds it per call, costing seconds of retrace/recompile)."""
    import jax
    import numpy as _np
    from jax.sharding import Mesh, PartitionSpec
    from jax.experimental.shard_map import shard_map
    from concourse import bass2jax, mybir

    bass2jax.install_neuronx_cc_hook()
    partition_name = nc.partition_id_tensor.name if nc.partition_id_tensor else None
    in_names, out_names, out_avals, zero_outs = [], [], [], []
    for alloc in nc.m.functions[0].allocations:
        if not isinstance(alloc, mybir.MemoryLocationSet):
            continue
        name = alloc.memorylocations[0].name
        if alloc.kind == "ExternalInput":
            if name != partition_name:
                in_names.append(name)
        elif alloc.kind == "ExternalOutput":
            shape = tuple(alloc.tensor_shape)
            dtype = mybir.dt.np(alloc.dtype)
            out_names.append(name)
            out_avals.append(jax.core.ShapedArray(shape, dtype))
            zero_outs.append(_np.zeros(shape, dtype))
    n_params = len(in_names)
    n_outs = len(out_avals)
    in_names_all = list(in_names) + list(out_names)
    if partition_name is not None:
        in_names_all.append(partition_name)

    def _body(*args):
        operands = list(args)
        if partition_name is not None:
            operands.append(bass2jax.partition_id_tensor())
        outs = bass2jax._bass_exec_p.bind(
            *operands,
            out_avals=tuple(out_avals),
            in_names=tuple(in_names_all),
            out_names=tuple(out_names),
            lowering_input_output_aliases=(),
            sim_require_finite=True,
            sim_require_nnan=True,
            nc=nc,
        )
        return tuple(outs)

    donate = tuple(range(n_params, n_params + n_outs))
    devices = jax.devices()[:NCORES]
    mesh = Mesh(_np.asarray(devices), ("core",))
    sharded = jax.jit(
        shard_map(_body, mesh=mesh,
                  in_specs=(PartitionSpec("core"),) * (n_params + n_outs),
                  out_specs=(PartitionSpec("core"),) * n_outs,
                  check_rep=False),
        donate_argnums=donate, keep_unused=True,
    )

    def run(in_maps):
        concat_in = [
            np.concatenate([np.asarray(in_maps[c][nm]) for c in range(NCORES)], axis=0)
            for nm in in_names[:n_params]
        ]
        concat_zeros = [np.zeros((NCORES * z.shape[0], *z.shape[1:]), z.dtype)
                        for z in zero_outs]
        out_arrs = sharded(*concat_in, *concat_zeros)
        return [
            {nm: np.asarray(out_arrs[i]).reshape(NCORES, *out_avals[i].shape)[c]
             for i, nm in enumerate(out_names)}
            for c in range(NCORES)
        ]

    run.sharded = sharded
    run.zero_outs = zero_outs
    run.in_names = in_names[:n_params]
    run.out_names = out_names
    run.out_avals = out_avals
    return run


_BUILT = {}


def kernel(**inputs):
    a = np.asarray(inputs["a"], np.float32)
    s_prev = np.asarray(inputs["s_prev"], np.float32)
    W1 = np.asarray(inputs["W1"], np.float32)
    b1 = np.asarray(inputs["b1"], np.float32)
    W2 = np.asarray(inputs["W2"], np.float32)
    b2 = np.asarray(inputs["b2"], np.float32)
    w_c = np.asarray(inputs["w_c"], np.float32)
    w_u = np.asarray(inputs["w_u"], np.float32)
    w_f = np.asarray(inputs["w_f"], np.float32)
    w_o = np.asarray(inputs["w_o"], np.float32)
    b_c = np.asarray(inputs["b_c"], np.float32)
    b_u = np.asarray(inputs["b_u"], np.float32)
    b_f = np.asarray(inputs["b_f"], np.float32)
    b_o = np.asarray(inputs["b_o"], np.float32)
    wo = int(np.asarray(inputs["word_output"]))

    if wo not in _BUILT:
        nc_ = _build(wo)
        _BUILT[wo] = (nc_, _make_runner(nc_))
    nc, runner = _BUILT[wo]

    W1aT = np.zeros((F, MIDP), np.float16)
    W1aT[:, :MID] = W1[:, :F].T
    W1sT = np.zeros((O, MIDP), np.float16)
    W1sT[:, :MID] = W1[:, F:].T
    W2p = np.zeros((MIDP,), np.float32)
    W2p[:MID] = W2[0]
    W2c = W2p.reshape(8, 128).T.astype(np.float16)
    b1p = np.zeros((MIDP,), np.float32)
    b1p[:MID] = b1
    b1T = b1p.reshape(8, 128).T.copy()
    WgT = np.concatenate([w.T for w in (w_c, w_u, w_f, w_o)], axis=1).astype(np.float16)
    bgv = np.concatenate([b_c, b_u, b_f, b_o]).astype(np.float32)  # [2048]
    # bgT128[p, g4*32+oc*8+bl] = bg[g4*512+oc*128+p]
    bgT128 = np.ascontiguousarray(
        bgv.reshape(4, 4, 128).transpose(2, 0, 1)[:, :, :, None]
        .repeat(8, 3).reshape(128, 128))
    common = {
        "W1aT": W1aT, "W1sT": W1sT, "W2c": W2c, "b1T": b1T,
        "b2bc": np.full((128, 1), float(b2.reshape(-1)[0]), np.float32),
        "WgT": WgT, "bgT128": bgT128,
    }
    in_maps = []
    for c in range(NCORES):
        b0 = c * BC
        ac = a[b0:b0 + BC]
        sc = s_prev[b0:b0 + BC]  # [16, 512]
        # sPT64[g, p, oc*8+bl] = s[g*8+bl, oc*128+p]
        sT64 = sc.reshape(2, 8, 4, 128).transpose(0, 3, 2, 1).reshape(2, 128, 32)
        in_maps.append({
            **common,
            "aT": np.ascontiguousarray(ac.transpose(2, 0, 1).reshape(F, BT)).astype(np.float16),
            "aN": np.ascontiguousarray(ac.reshape(BT, F)).astype(np.float16),
            "sPT64": np.ascontiguousarray(sT64).astype(np.float16),
        })

    results = None
    for attempt in range(4):
        try:
            results = runner(in_maps)
            break
        except Exception:
            if attempt == 3:
                raise
            import time as _time
            _time.sleep(1.0)
            if attempt >= 1:
                runner = _make_runner(nc)
                _BUILT[wo] = (nc, runner)
    global _LAST_RESULTS
    _LAST_RESULTS = results
    out = np.empty((B, wo, O), np.float32)
    for c in range(NCORES):
        arr = results[c]["out"]  # [wo, 2, 128, 32] with col = oc*8+bl
        out[c * BC:(c + 1) * BC] = (
            arr.reshape(wo, 2, 128, 4, 8).transpose(1, 4, 0, 3, 2).reshape(BC, wo, O)
        )
    return out
